# revision 5
# baseline (speedup 1.0000x reference)
"""Trainium2 Bass kernel for a single-head attention module (v4).

reference math (fp32):
    q = x @ Wq + bq; k = x @ Wk + bk; v = x @ Wv + bv        # [B,S,64]
    scores = (q @ k.T) / sqrt(S)                             # [B,S,S]
    scores = where(mask, -1e9, scores)
    out = softmax(scores, -1) @ v                            # [B,S,64]

Sharding: 8 cores = (batch b = c//2) x (sequence half h = c%2); each core
owns 1024 rows; pairs exchange K.T and V' via pairwise AllGathers. Key
order is host-rotated to [my keys, partner keys] so local attention
never waits on the exchange.

v4 layout/engine plan:
- Host supplies x pre-transposed ([DIN, H] bf16 — layout prep, like the
  mask rotation) and all small constants packed into ONE [128, CB] byte
  block (a single DMA; HWDGE slots are 625ns each and serialize).
- bk is dropped: (q+bq)@bk is constant per query and cancels in the
  softmax normalization.  One [Wq'|Wk] stationary pass projects Q.T and
  K.T together; a full-width DVE tensor_scalar_add applies bq while
  copying psum->sbuf (K rows +0), and the K.T half moves to a base-0
  tile via SBUF->SBUF DMA (engines cannot cross partitions; DMA can).
  1/sqrt(S) is folded into Wq'/bq' on host.
- V is computed in natural [key, d] layout (x.T chunks stationary, Wv
  moving) so V' needs no transposes; bv enters via rank-1 ones-row
  matmuls; V' carries a ones-column that makes the C' matmul produce
  the softmax denominator for free.
- Attention per 128-key chunk: f32r score matmuls into a 2-bank psum,
  exp on ACT (psum->sbuf bf16), u8 keep-mask multiply, C'-accumulate
  matmuls (V' stationary bf16, P.T moving bf16).  Chunks 0-3 run
  512-wide on query group 0 while group 1's x still streams in
  (leading singles), with the group-1 halves appended at the end
  (trailing singles); those single mask-multiplies run on Pool, which
  idles at both ends, while DVE takes the steady-state ones.
- The SP DMA queue is hand-ordered so the single DMA-device FIFO serves
  transfers roughly in deadline order: x group 0, mask 0-1, first half
  of x group 1, kt0 (ahead of the last x pairs), mask 2-3, the split
  exchange stages (kt0, kt1, V'), and the remaining mask chunks.
- Finalize at 128-query granularity (DVE copy, PE transpose, DVE
  reciprocal, ACT multiply), one output DMA per query group; group 0's
  finalize overlaps the trailing singles.
"""

import numpy as np
import ml_dtypes

import concourse.bass as bass
import concourse.mybir as mybir
import concourse.tile as tile
from concourse import bacc
from concourse.bass_utils import run_bass_kernel_spmd
from concourse.masks import make_identity
from concourse.tile import add_dep_helper

B, S, DIN, DOUT = 4, 2048, 1024, 64
H = S // 2          # rows (queries/keys) owned per core
P = 128             # partitions
NF = DIN // P       # 8 feature chunks
NS = S // P         # 16 key chunks (rotated order: 0-7 local, 8-15 partner)
QC = 512            # queries per projection group / matmul moving limit
NQC = H // QC       # 2 query groups
DP = DOUT + 1       # V' columns (V plus ones-column)
SINGLES = 0         # leading local chunks exp'd 512-wide per query group
KTG_BY = DOUT * QC * 4      # bytes of one K.T group (kept f32r)
VP_BY = P * NF * DP * 2     # bytes of local V' (bf16)
# packed constant block: wqk | wv | ball | pit | bv (bytes per partition)
CB_WQK = NF * P * 2
CB_WV = NF * DOUT * 2
CB_BALL = 4
CB_PIT = 4
CB_BV = DOUT * 2
CB = CB_WQK + CB_WV + CB_BALL + CB_PIT + CB_BV

F32 = mybir.dt.float32
F32R = mybir.dt.float32r
BF16 = mybir.dt.bfloat16
U8 = mybir.dt.uint8

N_CORES = 8
PAIRS = [[0, 1], [2, 3], [4, 5], [6, 7]]


def _pool_mask_half(ci, n, single):
    """True -> mask multiply for this (chunk, group) half runs on Pool.
    DVE gets one 594ns multiply per chunk (n=0) so it never outpaces the
    1038ns exp period; Pool takes most n=1 halves; every 4th chunk's n=1
    stays on DVE so Pool (1111ns/op) does not accumulate a backlog that
    would delay the final C' accumulations."""
    return n == 1 and ci % 4 != 3 and ci != 14


def build_attention_nc(unroll: int = 1, fake_cc: bool = False):
    nc = bacc.Bacc("TRN2", target_bir_lowering=False, debug=False,
                   num_devices=N_CORES)

    xt_d = nc.dram_tensor("xt", [DIN, H], BF16, kind="ExternalInput")
    nmt_d = nc.dram_tensor("nmt", [S, H], U8, kind="ExternalInput")
    cb_d = nc.dram_tensor("cb", [P, CB], U8, kind="ExternalInput")
    out_d = nc.dram_tensor("out", [H, DOUT], F32, kind="ExternalOutput")

    Exp = mybir.ActivationFunctionType.Exp

    with tile.TileContext(nc) as tc:
        with (
            tc.tile_pool(name="consts", bufs=1) as consts,
            tc.tile_pool(name="persist", bufs=1) as persist,
            tc.tile_pool(name="ptp", bufs=4) as ptp,
            tc.tile_pool(name="p2p", bufs=8) as p2p,
            tc.tile_pool(name="fin", bufs=4) as fin,
            tc.tile_pool(name="dramb", bufs=1, space="DRAM") as dramb,
            tc.tile_pool(name="st_ps", bufs=2, space="PSUM") as st_ps,
            tc.tile_pool(name="scr_ps", bufs=2, space="PSUM") as scr_ps,
            tc.tile_pool(name="cp_ps", bufs=1, space="PSUM") as cp_ps,
        ):
            # ---- packed constants (DMA emitted in the SP issue order) ------
            cbl = consts.tile([P, CB], U8, tag="cbl")
            o0 = 0
            wqk = cbl[:, o0:o0 + CB_WQK].bitcast(BF16).rearrange(
                "p (c d) -> p c d", d=P)
            o0 += CB_WQK
            wv = cbl[:, o0:o0 + CB_WV].bitcast(BF16).rearrange(
                "p (c d) -> p c d", d=DOUT)
            o0 += CB_WV
            ball = cbl[:, o0:o0 + CB_BALL].bitcast(F32)
            o0 += CB_BALL
            pit = cbl[0:1, o0:o0 + CB_PIT].bitcast(mybir.dt.uint32)
            o0 += CB_PIT
            bvrow = cbl[0:1, o0:o0 + CB_BV].bitcast(BF16)
            ones = consts.tile([1, P], BF16, tag="ones")
            nc.vector.memset(ones, 1.0)
            ident = consts.tile([P, P], F32, tag="ident")
            make_identity(nc, ident)
            # PE warmup: serial transpose chain ramps the tensor engine
            # p-state before the first projection matmuls
            pwarm = scr_ps.tile([P, QC], F32, tag="scr")
            for _ in range(17):
                nc.tensor.transpose(pwarm[:, :P], ident, ident)
            # preload the ACT Exp table so the first real exp skips the
            # 1283ns table load
            wtiny = consts.tile([1, 1], F32, tag="wtiny")
            nc.scalar.activation(out=wtiny, in_=ident[0:1, 0:1], func=Exp)

            for u in range(unroll):
                xt = persist.tile([P, NF, H], BF16, tag="xt", name="xt")
                nm8 = persist.tile([P, NS, H], U8, tag="m8", name="m8")
                qk = [
                    persist.tile([P, QC], F32R, tag=f"qk{g}", name=f"qk{g}")
                    for g in range(NQC)
                ]
                kt = [
                    persist.tile([DOUT, QC], F32R, tag=f"kt{g}", name=f"kt{g}")
                    for g in range(NQC)
                ]
                ktp = [
                    persist.tile([DOUT, QC], F32R, tag=f"ktp{g}",
                                 name=f"ktp{g}")
                    for g in range(NQC)
                ]
                vp = persist.tile([P, NF, DP], BF16, tag="vp", name="vp")
                vpp = persist.tile([P, NF, DP], BF16, tag="vpp", name="vpp")
                exi = [
                    dramb.tile([1, KTG_BY], U8, tag=f"exi{g}", name=f"exi{g}")
                    for g in range(NQC)
                ]
                exo = [
                    dramb.tile([2, KTG_BY], U8, tag=f"exo{g}", name=f"exo{g}")
                    for g in range(NQC)
                ]
                exvi = dramb.tile([1, VP_BY], U8, tag="exvi", name="exvi")
                exvo = dramb.tile([2, VP_BY], U8, tag="exvo", name="exvo")

                def xload(g, fp):
                    return nc.sync.dma_start(
                        out=xt[:, 2 * fp:2 * fp + 2, g * QC:(g + 1) * QC],
                        in_=xt_d.ap()[2 * fp * P:(2 * fp + 2) * P,
                                      g * QC:(g + 1) * QC].rearrange(
                            "(c p) s -> p c s", p=P),
                    )

                def mask_dma(eng, lo, hi):
                    return eng.dma_start(
                        out=nm8[:, lo:hi, :],
                        in_=nmt_d.ap()[lo * P:hi * P, :].rearrange(
                            "(c p) q -> p c q", p=P),
                    )

                def project_qk(g):
                    """[Wq'|Wk] pass for one 512-row group."""
                    pqk = scr_ps.tile([P, QC], F32, tag="scr")
                    for cf in range(NF):
                        nc.tensor.matmul(
                            pqk, wqk[:, cf], xt[:, cf, g * QC:(g + 1) * QC],
                            start=(cf == 0), stop=(cf == NF - 1),
                        )
                    # full-width copy applies bq (K rows get +0); on ACT,
                    # which idles until the first exp
                    nc.scalar.activation(
                        out=qk[g], in_=pqk,
                        func=mybir.ActivationFunctionType.Identity,
                        bias=ball)

                def project_v(g):
                    pv = scr_ps.tile([P, QC], F32, tag="scr")
                    for sb in range(4 * g, 4 * (g + 1)):
                        o = (sb - 4 * g) * DOUT
                        for cf in range(NF):
                            nc.tensor.matmul(
                                pv[:, o:o + DOUT],
                                xt[:, cf, sb * P:(sb + 1) * P],
                                wv[:, cf],
                                start=(cf == 0), stop=False,
                            )
                        nc.tensor.matmul(
                            pv[:, o:o + DOUT], ones, bvrow,
                            start=False, stop=True,
                        )
                    if g == 0:
                        nc.vector.memset(vp, 1.0)
                    nc.vector.tensor_copy(
                        out=vp[:, 4 * g:4 * (g + 1), :DOUT],
                        in_=pv[:, :4 * DOUT].rearrange(
                            "p (c d) -> p c d", d=DOUT),
                    )

                def exchange_kt(g):
                    nc.sync.dma_start(
                        out=exi[g][0:1, :].bitcast(F32R)
                        .rearrange("one (k s) -> k (one s)", k=DOUT),
                        in_=qk[g][DOUT:, :],
                    )
                    if fake_cc:
                        nc.sync.dma_start(out=exo[g][0], in_=exi[g][0])
                        nc.sync.dma_start(out=exo[g][1], in_=exi[g][0])
                    else:
                        nc.gpsimd.collective_compute(
                            "AllGather", mybir.AluOpType.bypass,
                            replica_groups=PAIRS,
                            ins=[exi[g][:]], outs=[exo[g][:]],
                        )

                def readback_kt(g):
                    nc.sync.dma_start(
                        out=ktp[g],
                        in_=exo[g][:].bitcast(F32R)
                        .rearrange("two (k s) -> two k s", k=DOUT)
                        [bass.ds(prv, 1), :, :]
                        .rearrange("one k s -> k (one s)"),
                    )

                # ---- attention emission helpers ---------------------------
                def chunk_views(ci):
                    if ci < NS // 2:
                        g, kb = ci // 4, (ci % 4) * P
                        return kt[g][:, kb:kb + P], vp[:, ci, :]
                    g, kb = (ci - 8) // 4, (ci % 4) * P
                    return ktp[g][:, kb:kb + P], vpp[:, ci - 8, :]

                cps = [
                    cp_ps.tile([DP, QC], F32, tag=f"cp{n}", name=f"cp{n}")
                    for n in range(NQC)
                ]
                sched = [(ci, (0,), True) for ci in range(SINGLES)]
                sched += [(ci, (0, 1), False) for ci in range(SINGLES, NS)]
                sched += [(ci, (1,), True) for ci in range(SINGLES)]
                first = {}
                last = {}
                for pos, (ci, n_list, single) in enumerate(sched):
                    for n in n_list:
                        first.setdefault(n, pos)
                        last[n] = pos

                def emit_attention(lo, hi):
                    # C' matmuls are emitted two chunks behind the score
                    # matmuls so the PE queue never stalls the exp stream on
                    # a late mask multiply or V'/K.T readback
                    pending = []

                    def flush_cprime():
                        pos, n, vp_sl, p2 = pending.pop(0)
                        nc.tensor.matmul(
                            cps[n], vp_sl, p2,
                            start=(pos == first[n]), stop=(pos == last[n]),
                        )

                    for pos in range(lo, hi):
                        ci, n_list, single = sched[pos]
                        kt_sl, vp_sl = chunk_views(ci)
                        st = st_ps.tile([P, H], F32, tag="st")
                        for n in n_list:
                            nc.tensor.matmul(
                                st[:, n * QC:(n + 1) * QC], kt_sl,
                                qk[n][:DOUT, :],
                                start=True, stop=True,
                            )
                        pt = ptp.tile([P, H], BF16, tag="pt")
                        if single:
                            n = n_list[0]
                            nc.scalar.activation(
                                out=pt[:, :QC],
                                in_=st[:, n * QC:(n + 1) * QC], func=Exp)
                        else:
                            nc.scalar.activation(out=pt, in_=st, func=Exp)
                        for n in n_list:
                            psl = slice(0, QC) if single else slice(
                                n * QC, (n + 1) * QC)
                            p2 = p2p.tile([P, QC], BF16, tag=f"p2_{n}",
                                          name=f"p2_{n}")
                            eng = (nc.gpsimd if _pool_mask_half(ci, n, single)
                                   else nc.vector)
                            eng.tensor_mul(p2, pt[:, psl],
                                           nm8[:, ci, n * QC:(n + 1) * QC])
                            pending.append((pos, n, vp_sl, p2))
                        while len(pending) > 8:
                            flush_cprime()
                    while pending:
                        flush_cprime()

                # ---- issue order (SP queue == DMA deadline order) ---------
                xload(0, 0)
                if u == 0:
                    nc.sync.dma_start(out=cbl, in_=cb_d.ap())
                    pregs = nc.alloc_registers()
                for fp in range(1, NF // 2):
                    xload(0, fp)
                for fp in range(NF // 2):
                    xload(1, fp)
                project_qk(0)
                project_v(0)
                nc.sync.dma_start(out=kt[0], in_=qk[0][DOUT:, :])
                mask_dma(nc.sync, 0, 2)
                mask_dma(nc.sync, 2, 4)
                if u == 0:
                    nc.regs_load(pregs, pit)
                    prv = nc.snap(pregs)
                project_qk(1)
                project_v(1)
                nc.sync.dma_start(out=kt[1], in_=qk[1][DOUT:, :])
                nc.sync.dma_start(
                    out=exi[0][0:1, :].bitcast(F32R)
                    .rearrange("one (k s) -> k (one s)", k=DOUT),
                    in_=qk[0][DOUT:, :],
                )
                if fake_cc:
                    nc.sync.dma_start(out=exo[0][0], in_=exi[0][0])
                    nc.sync.dma_start(out=exo[0][1], in_=exi[0][0])
                else:
                    nc.gpsimd.collective_compute(
                        "AllGather", mybir.AluOpType.bypass,
                        replica_groups=PAIRS,
                        ins=[exi[0][:]], outs=[exo[0][:]],
                    )
                mask_dma(nc.sync, 4, 6)
                nc.sync.dma_start(
                    out=exi[1][0:1, :].bitcast(F32R)
                    .rearrange("one (k s) -> k (one s)", k=DOUT),
                    in_=qk[1][DOUT:, :],
                )
                readback_kt(0)
                mask_dma(nc.sync, 6, 8)
                mask_dma(nc.sync, 8, 10)
                nc.sync.dma_start(
                    out=exvi[0:1, :].bitcast(BF16).rearrange(
                        "one (p d) -> p (one d)", p=P),
                    in_=vp[:].rearrange("p c d -> p (c d)"),
                )
                if fake_cc:
                    nc.sync.dma_start(out=exvo[0], in_=exvi[0])
                    nc.sync.dma_start(out=exvo[1], in_=exvi[0])
                    nc.sync.dma_start(out=exo[1][0], in_=exi[1][0])
                    nc.sync.dma_start(out=exo[1][1], in_=exi[1][0])
                else:
                    nc.gpsimd.collective_compute(
                        "AllGather", mybir.AluOpType.bypass,
                        replica_groups=PAIRS,
                        ins=[exvi[:]], outs=[exvo[:]],
                    )
                    nc.gpsimd.collective_compute(
                        "AllGather", mybir.AluOpType.bypass,
                        replica_groups=PAIRS,
                        ins=[exi[1][:]], outs=[exo[1][:]],
                    )
                nc.sync.dma_start(
                    out=vpp[:].rearrange("p c d -> p (c d)"),
                    in_=exvo[:].bitcast(BF16)
                    .rearrange("two (p d) -> two p d", p=P)
                    [bass.ds(prv, 1), :, :]
                    .rearrange("one p d -> p (one d)"),
                )
                mask_dma(nc.sync, 10, 12)
                readback_kt(1)
                mask_dma(nc.sync, 12, NS)
                emit_attention(SINGLES, len(sched))

                # ---- finalize: one copy + one DMA per group ---------------
                for n in range(NQC):
                    ct = fin.tile([DP, QC], F32, tag="ct")
                    nc.vector.tensor_copy(out=ct, in_=cps[n])
                    c_sb = fin.tile([P, QC // P, DOUT], F32, tag="c_sb")
                    for qb in range(QC // P):
                        tp = scr_ps.tile([P, QC], F32, tag="scr")
                        nc.tensor.transpose(
                            tp[:, :DP], ct[:, qb * P:(qb + 1) * P],
                            ident[:DP, :DP])
                        rec = fin.tile([P, 1], F32, tag="rec")
                        nc.vector.reciprocal(rec, tp[:, DOUT:DP])
                        nc.scalar.mul(c_sb[:, qb, :], tp[:, :DOUT], rec)
                    nc.sync.dma_start(
                        out=out_d.ap()[n * QC:(n + 1) * QC, :].rearrange(
                            "(c p) d -> p c d", p=P),
                        in_=c_sb,
                    )

    nc.compile()
    return nc


def shard_inputs(inputs):
    """Full inputs -> per-core in_maps (list of 8 dicts)."""
    bf = ml_dtypes.bfloat16
    x = np.asarray(inputs["input_tensor"], dtype=np.float32)
    m = np.asarray(inputs["attention_mask"])
    nm = (~m).view(np.uint8) if m.dtype == np.bool_ else (m == 0).astype(np.uint8)

    scale = np.float32(np.sqrt(np.float32(S)))
    wq = np.asarray(inputs["Wq"], np.float32) / scale
    bq = np.asarray(inputs["bq"], np.float32) / scale
    wk = np.asarray(inputs["Wk"], np.float32)
    # bk is omitted: it only shifts scores by a per-query constant, which
    # softmax normalization cancels.
    wqk_b = (np.concatenate([wq, wk], axis=1).astype(bf)
             .reshape(NF, P, P).transpose(1, 0, 2).reshape(P, NF * P))
    wv_b = (np.asarray(inputs["Wv"], np.float32).astype(bf)
            .reshape(NF, P, DOUT).transpose(1, 0, 2).reshape(P, NF * DOUT))
    ball_b = np.concatenate([bq, np.zeros(DOUT, np.float32)]).astype(
        np.float32)[:, None]
    bv_b = np.asarray(inputs["bv"], np.float32).astype(bf)
    com_base = np.zeros((P, CB), dtype=np.uint8)
    o = 0
    com_base[:, o:o + CB_WQK] = wqk_b.view(np.uint8); o += CB_WQK
    com_base[:, o:o + CB_WV] = wv_b.view(np.uint8); o += CB_WV
    com_base[:, o:o + CB_BALL] = ball_b.view(np.uint8); o += CB_BALL
    o_pit = o; o += CB_PIT
    com_base[0, o:o + CB_BV] = bv_b.view(np.uint8); o += CB_BV

    in_maps = []
    for c in range(N_CORES):
        b, h = c // 2, c % 2
        qsl = slice(h * H, (h + 1) * H)
        # key order rotated per core: [my 1024 keys, partner's 1024]
        nmT = nm[b, qsl, :].T
        nmt = np.concatenate([nmT[h * H:(h + 1) * H],
                              nmT[(1 - h) * H:(2 - h) * H]], axis=0)
        cb = com_base.copy()
        cb[0, o_pit:o_pit + CB_PIT] = np.array(
            [1 - h], dtype=np.uint32).view(np.uint8)
        in_maps.append({
            "xt": np.ascontiguousarray(x[b, qsl].T.astype(bf)),
            "nmt": np.ascontiguousarray(nmt),
            "cb": cb,
        })
    return in_maps


_NC_CACHE = {}


def _get_nc(unroll: int = 1, fake_cc: bool = False):
    key = (unroll, fake_cc)
    if key not in _NC_CACHE:
        _NC_CACHE[key] = build_attention_nc(unroll, fake_cc)
    return _NC_CACHE[key]


def kernel(**inputs) -> np.ndarray:
    nc = _get_nc()
    in_maps = shard_inputs(inputs)
    res = run_bass_kernel_spmd(nc, in_maps, core_ids=list(range(N_CORES)))
    out = np.empty((B, S, DOUT), dtype=np.float32)
    for c in range(N_CORES):
        b, h = c // 2, c % 2
        out[b, h * H:(h + 1) * H] = res.results[c]["out"]
    return out


# revision 6
# speedup vs baseline: 1.0430x; 1.0430x over previous
"""Trainium2 Bass kernel for a single-head attention module (v4).

reference math (fp32):
    q = x @ Wq + bq; k = x @ Wk + bk; v = x @ Wv + bv        # [B,S,64]
    scores = (q @ k.T) / sqrt(S)                             # [B,S,S]
    scores = where(mask, -1e9, scores)
    out = softmax(scores, -1) @ v                            # [B,S,64]

Sharding: 8 cores = (batch b = c//2) x (sequence half h = c%2); each core
owns 1024 rows; pairs exchange K.T and V' via pairwise AllGathers. Key
order is host-rotated to [my keys, partner keys] so local attention
never waits on the exchange.

v4 layout/engine plan:
- Host supplies x pre-transposed ([DIN, H] bf16 — layout prep, like the
  mask rotation) and all small constants packed into ONE [128, CB] byte
  block (a single DMA; HWDGE slots are 625ns each and serialize).
- bk is dropped: (q+bq)@bk is constant per query and cancels in the
  softmax normalization.  One [Wq'|Wk] stationary pass projects Q.T and
  K.T together; a full-width DVE tensor_scalar_add applies bq while
  copying psum->sbuf (K rows +0), and the K.T half moves to a base-0
  tile via SBUF->SBUF DMA (engines cannot cross partitions; DMA can).
  1/sqrt(S) is folded into Wq'/bq' on host.
- V is computed in natural [key, d] layout (x.T chunks stationary, Wv
  moving) so V' needs no transposes; bv enters via rank-1 ones-row
  matmuls; V' carries a ones-column that makes the C' matmul produce
  the softmax denominator for free.
- Attention per 128-key chunk: f32r score matmuls into a 2-bank psum,
  exp on ACT (psum->sbuf bf16), u8 keep-mask multiply, C'-accumulate
  matmuls (V' stationary bf16, P.T moving bf16), emitted two chunks
  behind the score matmuls so the PE queue never stalls the exp stream
  on a late mask multiply or readback.  DVE takes one multiply per
  chunk (n=0 plus every 4th n=1) so it never outpaces the 1038ns exp
  period; Pool absorbs the rest.
- The SP DMA queue is hand-ordered so the single DMA-device FIFO serves
  transfers roughly in deadline order: x group 0, mask 0-1, first half
  of x group 1, kt0 (ahead of the last x pairs), mask 2-3, the split
  exchange stages (kt0, kt1, V'), and the remaining mask chunks.
- Finalize per query group: one DVE psum->sbuf copy, then per-128-query
  PE transpose, DVE reciprocal of the denominator column, ACT multiply
  (idle after the last exp), one output DMA per group.
"""

import numpy as np
import ml_dtypes

import concourse.bass as bass
import concourse.mybir as mybir
import concourse.tile as tile
from concourse import bacc
from concourse.bass_utils import run_bass_kernel_spmd
from concourse.masks import make_identity
from concourse.tile import add_dep_helper

B, S, DIN, DOUT = 4, 2048, 1024, 64
H = S // 2          # rows (queries/keys) owned per core
P = 128             # partitions
NF = DIN // P       # 8 feature chunks
NS = S // P         # 16 key chunks (rotated order: 0-7 local, 8-15 partner)
QC = 512            # queries per projection group / matmul moving limit
NQC = H // QC       # 2 query groups
DP = DOUT + 1       # V' columns (V plus ones-column)
SINGLES = 0         # leading local chunks exp'd 512-wide per query group
KTG_BY = DOUT * QC * 4      # bytes of one K.T group (kept f32r)
VP_BY = P * NF * DP * 2     # bytes of local V' (bf16)
# packed constant block: wqk | wv | ball | pit | bv (bytes per partition)
CB_WQK = NF * P * 2
CB_WV = NF * DOUT * 2
CB_BALL = 4
CB_PIT = 4
CB_BV = DOUT * 2
CB = CB_WQK + CB_WV + CB_BALL + CB_PIT + CB_BV

F32 = mybir.dt.float32
F32R = mybir.dt.float32r
BF16 = mybir.dt.bfloat16
U8 = mybir.dt.uint8

N_CORES = 8
PAIRS = [[0, 1], [2, 3], [4, 5], [6, 7]]


def _pool_mask_half(ci, n, single):
    """True -> mask multiply for this (chunk, group) half runs on Pool.
    DVE gets one 594ns multiply per chunk (n=0) so it never outpaces the
    1038ns exp period; Pool takes most n=1 halves; every 4th chunk's n=1
    stays on DVE so Pool (1111ns/op) does not accumulate a backlog that
    would delay the final C' accumulations."""
    return n == 1 and ci % 4 != 3 and ci != 14


def build_attention_nc(unroll: int = 1, fake_cc: bool = False):
    nc = bacc.Bacc("TRN2", target_bir_lowering=False, debug=False,
                   num_devices=N_CORES)

    xt_d = nc.dram_tensor("xt", [DIN, H], BF16, kind="ExternalInput")
    nmt_d = nc.dram_tensor("nmt", [S, H], U8, kind="ExternalInput")
    cb_d = nc.dram_tensor("cb", [P, CB], U8, kind="ExternalInput")
    out_d = nc.dram_tensor("out", [H, DOUT], F32, kind="ExternalOutput")

    Exp = mybir.ActivationFunctionType.Exp

    with tile.TileContext(nc) as tc:
        with (
            tc.tile_pool(name="consts", bufs=1) as consts,
            tc.tile_pool(name="persist", bufs=1) as persist,
            tc.tile_pool(name="ptp", bufs=4) as ptp,
            tc.tile_pool(name="p2p", bufs=8) as p2p,
            tc.tile_pool(name="fin", bufs=4) as fin,
            tc.tile_pool(name="dramb", bufs=1, space="DRAM") as dramb,
            tc.tile_pool(name="st_ps", bufs=2, space="PSUM") as st_ps,
            tc.tile_pool(name="scr_ps", bufs=2, space="PSUM") as scr_ps,
            tc.tile_pool(name="cp_ps", bufs=1, space="PSUM") as cp_ps,
        ):
            # ---- packed constants (DMA emitted in the SP issue order) ------
            cbl = consts.tile([P, CB], U8, tag="cbl")
            o0 = 0
            wqk = cbl[:, o0:o0 + CB_WQK].bitcast(BF16).rearrange(
                "p (c d) -> p c d", d=P)
            o0 += CB_WQK
            wv = cbl[:, o0:o0 + CB_WV].bitcast(BF16).rearrange(
                "p (c d) -> p c d", d=DOUT)
            o0 += CB_WV
            ball = cbl[:, o0:o0 + CB_BALL].bitcast(F32)
            o0 += CB_BALL
            pit = cbl[0:1, o0:o0 + CB_PIT].bitcast(mybir.dt.uint32)
            o0 += CB_PIT
            bvrow = cbl[0:1, o0:o0 + CB_BV].bitcast(BF16)
            ones = consts.tile([1, P], BF16, tag="ones")
            nc.vector.memset(ones, 1.0)
            ident = consts.tile([P, P], F32, tag="ident")
            make_identity(nc, ident)
            # PE warmup: serial transpose chain ramps the tensor engine
            # p-state before the first projection matmuls
            pwarm = scr_ps.tile([P, QC], F32, tag="scr")
            for _ in range(17):
                nc.tensor.transpose(pwarm[:, :P], ident, ident)
            # preload the ACT Exp table so the first real exp skips the
            # 1283ns table load
            wtiny = consts.tile([1, 1], F32, tag="wtiny")
            nc.scalar.activation(out=wtiny, in_=ident[0:1, 0:1], func=Exp)

            for u in range(unroll):
                xt = persist.tile([P, NF, H], BF16, tag="xt", name="xt")
                nm8 = persist.tile([P, NS, H], U8, tag="m8", name="m8")
                qk = [
                    persist.tile([P, QC], F32R, tag=f"qk{g}", name=f"qk{g}")
                    for g in range(NQC)
                ]
                kt = [
                    persist.tile([DOUT, QC], F32R, tag=f"kt{g}", name=f"kt{g}")
                    for g in range(NQC)
                ]
                ktp = [
                    persist.tile([DOUT, QC], F32R, tag=f"ktp{g}",
                                 name=f"ktp{g}")
                    for g in range(NQC)
                ]
                vp = persist.tile([P, NF, DP], BF16, tag="vp", name="vp")
                vpp = persist.tile([P, NF, DP], BF16, tag="vpp", name="vpp")
                exi = [
                    dramb.tile([1, KTG_BY], U8, tag=f"exi{g}", name=f"exi{g}")
                    for g in range(NQC)
                ]
                exo = [
                    dramb.tile([2, KTG_BY], U8, tag=f"exo{g}", name=f"exo{g}")
                    for g in range(NQC)
                ]
                exvi = dramb.tile([1, VP_BY], U8, tag="exvi", name="exvi")
                exvo = dramb.tile([2, VP_BY], U8, tag="exvo", name="exvo")

                def xload(g, fp):
                    return nc.sync.dma_start(
                        out=xt[:, 2 * fp:2 * fp + 2, g * QC:(g + 1) * QC],
                        in_=xt_d.ap()[2 * fp * P:(2 * fp + 2) * P,
                                      g * QC:(g + 1) * QC].rearrange(
                            "(c p) s -> p c s", p=P),
                    )

                def mask_dma(eng, lo, hi):
                    return eng.dma_start(
                        out=nm8[:, lo:hi, :],
                        in_=nmt_d.ap()[lo * P:hi * P, :].rearrange(
                            "(c p) q -> p c q", p=P),
                    )

                def project_qk(g):
                    """[Wq'|Wk] pass for one 512-row group."""
                    pqk = scr_ps.tile([P, QC], F32, tag="scr")
                    for cf in range(NF):
                        nc.tensor.matmul(
                            pqk, wqk[:, cf], xt[:, cf, g * QC:(g + 1) * QC],
                            start=(cf == 0), stop=(cf == NF - 1),
                        )
                    # full-width copy applies bq (K rows get +0); on ACT,
                    # which idles until the first exp
                    nc.scalar.activation(
                        out=qk[g], in_=pqk,
                        func=mybir.ActivationFunctionType.Identity,
                        bias=ball)

                def project_v(g):
                    pv = scr_ps.tile([P, QC], F32, tag="scr")
                    for sb in range(4 * g, 4 * (g + 1)):
                        o = (sb - 4 * g) * DOUT
                        for cf in range(NF):
                            nc.tensor.matmul(
                                pv[:, o:o + DOUT],
                                xt[:, cf, sb * P:(sb + 1) * P],
                                wv[:, cf],
                                start=(cf == 0), stop=False,
                            )
                        nc.tensor.matmul(
                            pv[:, o:o + DOUT], ones, bvrow,
                            start=False, stop=True,
                        )
                    if g == 0:
                        nc.vector.memset(vp, 1.0)
                    nc.vector.tensor_copy(
                        out=vp[:, 4 * g:4 * (g + 1), :DOUT],
                        in_=pv[:, :4 * DOUT].rearrange(
                            "p (c d) -> p c d", d=DOUT),
                    )

                def exchange_kt(g):
                    nc.sync.dma_start(
                        out=exi[g][0:1, :].bitcast(F32R)
                        .rearrange("one (k s) -> k (one s)", k=DOUT),
                        in_=qk[g][DOUT:, :],
                    )
                    if fake_cc:
                        nc.sync.dma_start(out=exo[g][0], in_=exi[g][0])
                        nc.sync.dma_start(out=exo[g][1], in_=exi[g][0])
                    else:
                        nc.gpsimd.collective_compute(
                            "AllGather", mybir.AluOpType.bypass,
                            replica_groups=PAIRS,
                            ins=[exi[g][:]], outs=[exo[g][:]],
                        )

                def readback_kt(g):
                    nc.sync.dma_start(
                        out=ktp[g],
                        in_=exo[g][:].bitcast(F32R)
                        .rearrange("two (k s) -> two k s", k=DOUT)
                        [bass.ds(prv, 1), :, :]
                        .rearrange("one k s -> k (one s)"),
                    )

                # ---- attention emission helpers ---------------------------
                def chunk_views(ci):
                    if ci < NS // 2:
                        g, kb = ci // 4, (ci % 4) * P
                        return kt[g][:, kb:kb + P], vp[:, ci, :]
                    g, kb = (ci - 8) // 4, (ci % 4) * P
                    return ktp[g][:, kb:kb + P], vpp[:, ci - 8, :]

                cps = [
                    cp_ps.tile([DP, QC], F32, tag=f"cp{n}", name=f"cp{n}")
                    for n in range(NQC)
                ]
                sched = [(ci, (0,), True) for ci in range(SINGLES)]
                sched += [(ci, (0, 1), False) for ci in range(SINGLES, NS)]
                sched += [(ci, (1,), True) for ci in range(SINGLES)]
                first = {}
                last = {}
                for pos, (ci, n_list, single) in enumerate(sched):
                    for n in n_list:
                        first.setdefault(n, pos)
                        last[n] = pos

                def emit_attention(lo, hi):
                    # C' matmuls are emitted two chunks behind the score
                    # matmuls so the PE queue never stalls the exp stream on
                    # a late mask multiply or V'/K.T readback
                    pending = []

                    def flush_cprime():
                        pos, n, vp_sl, p2 = pending.pop(0)
                        nc.tensor.matmul(
                            cps[n], vp_sl, p2,
                            start=(pos == first[n]), stop=(pos == last[n]),
                        )

                    for pos in range(lo, hi):
                        ci, n_list, single = sched[pos]
                        kt_sl, vp_sl = chunk_views(ci)
                        st = st_ps.tile([P, H], F32, tag="st")
                        for n in n_list:
                            nc.tensor.matmul(
                                st[:, n * QC:(n + 1) * QC], kt_sl,
                                qk[n][:DOUT, :],
                                start=True, stop=True,
                            )
                        pt = ptp.tile([P, H], BF16, tag="pt")
                        if single:
                            n = n_list[0]
                            nc.scalar.activation(
                                out=pt[:, :QC],
                                in_=st[:, n * QC:(n + 1) * QC], func=Exp)
                        else:
                            nc.scalar.activation(out=pt, in_=st, func=Exp)
                        for n in n_list:
                            psl = slice(0, QC) if single else slice(
                                n * QC, (n + 1) * QC)
                            p2 = p2p.tile([P, QC], BF16, tag=f"p2_{n}",
                                          name=f"p2_{n}")
                            eng = (nc.gpsimd if _pool_mask_half(ci, n, single)
                                   else nc.vector)
                            eng.tensor_mul(p2, pt[:, psl],
                                           nm8[:, ci, n * QC:(n + 1) * QC])
                            pending.append((pos, n, vp_sl, p2))
                        while len(pending) > 8:
                            flush_cprime()
                    while pending:
                        flush_cprime()

                # ---- issue order (SP queue == DMA deadline order) ---------
                xload(0, 0)
                if u == 0:
                    nc.sync.dma_start(out=cbl, in_=cb_d.ap())
                    pregs = nc.alloc_registers()
                for fp in range(1, NF // 2):
                    xload(0, fp)
                for fp in range(NF // 2):
                    xload(1, fp)
                project_qk(0)
                project_v(0)
                nc.sync.dma_start(out=kt[0], in_=qk[0][DOUT:, :])
                mask_dma(nc.sync, 0, 2)
                mask_dma(nc.sync, 2, 4)
                if u == 0:
                    nc.regs_load(pregs, pit)
                    prv = nc.snap(pregs)
                project_qk(1)
                project_v(1)
                nc.sync.dma_start(out=kt[1], in_=qk[1][DOUT:, :])
                nc.sync.dma_start(
                    out=exi[0][0:1, :].bitcast(F32R)
                    .rearrange("one (k s) -> k (one s)", k=DOUT),
                    in_=qk[0][DOUT:, :],
                )
                if fake_cc:
                    nc.sync.dma_start(out=exo[0][0], in_=exi[0][0])
                    nc.sync.dma_start(out=exo[0][1], in_=exi[0][0])
                else:
                    nc.gpsimd.collective_compute(
                        "AllGather", mybir.AluOpType.bypass,
                        replica_groups=PAIRS,
                        ins=[exi[0][:]], outs=[exo[0][:]],
                    )
                mask_dma(nc.sync, 4, 6)
                nc.sync.dma_start(
                    out=exi[1][0:1, :].bitcast(F32R)
                    .rearrange("one (k s) -> k (one s)", k=DOUT),
                    in_=qk[1][DOUT:, :],
                )
                readback_kt(0)
                mask_dma(nc.sync, 6, 8)
                mask_dma(nc.sync, 8, 10)
                nc.sync.dma_start(
                    out=exvi[0:1, :].bitcast(BF16).rearrange(
                        "one (p d) -> p (one d)", p=P),
                    in_=vp[:].rearrange("p c d -> p (c d)"),
                )
                if fake_cc:
                    nc.sync.dma_start(out=exvo[0], in_=exvi[0])
                    nc.sync.dma_start(out=exvo[1], in_=exvi[0])
                    nc.sync.dma_start(out=exo[1][0], in_=exi[1][0])
                    nc.sync.dma_start(out=exo[1][1], in_=exi[1][0])
                else:
                    nc.gpsimd.collective_compute(
                        "AllGather", mybir.AluOpType.bypass,
                        replica_groups=PAIRS,
                        ins=[exvi[:]], outs=[exvo[:]],
                    )
                    nc.gpsimd.collective_compute(
                        "AllGather", mybir.AluOpType.bypass,
                        replica_groups=PAIRS,
                        ins=[exi[1][:]], outs=[exo[1][:]],
                    )
                nc.sync.dma_start(
                    out=vpp[:].rearrange("p c d -> p (c d)"),
                    in_=exvo[:].bitcast(BF16)
                    .rearrange("two (p d) -> two p d", p=P)
                    [bass.ds(prv, 1), :, :]
                    .rearrange("one p d -> p (one d)"),
                )
                mask_dma(nc.sync, 10, 12)
                readback_kt(1)
                mask_dma(nc.sync, 12, NS)
                emit_attention(SINGLES, len(sched))

                # ---- finalize: one copy + one DMA per group ---------------
                for n in range(NQC):
                    ct = fin.tile([DP, QC], F32, tag="ct")
                    nc.vector.tensor_copy(out=ct, in_=cps[n])
                    c_sb = fin.tile([P, QC // P, DOUT], F32, tag="c_sb")
                    for qb in range(QC // P):
                        tp = scr_ps.tile([P, QC], F32, tag="scr")
                        nc.tensor.transpose(
                            tp[:, :DP], ct[:, qb * P:(qb + 1) * P],
                            ident[:DP, :DP])
                        rec = fin.tile([P, 1], F32, tag="rec")
                        nc.vector.reciprocal(rec, tp[:, DOUT:DP])
                        nc.scalar.mul(c_sb[:, qb, :], tp[:, :DOUT], rec)
                    nc.sync.dma_start(
                        out=out_d.ap()[n * QC:(n + 1) * QC, :].rearrange(
                            "(c p) d -> p c d", p=P),
                        in_=c_sb,
                    )

    nc.compile()
    return nc


def shard_inputs(inputs):
    """Full inputs -> per-core in_maps (list of 8 dicts)."""
    bf = ml_dtypes.bfloat16
    x = np.asarray(inputs["input_tensor"], dtype=np.float32)
    m = np.asarray(inputs["attention_mask"])
    nm = (~m).view(np.uint8) if m.dtype == np.bool_ else (m == 0).astype(np.uint8)

    scale = np.float32(np.sqrt(np.float32(S)))
    wq = np.asarray(inputs["Wq"], np.float32) / scale
    bq = np.asarray(inputs["bq"], np.float32) / scale
    wk = np.asarray(inputs["Wk"], np.float32)
    # bk is omitted: it only shifts scores by a per-query constant, which
    # softmax normalization cancels.
    wqk_b = (np.concatenate([wq, wk], axis=1).astype(bf)
             .reshape(NF, P, P).transpose(1, 0, 2).reshape(P, NF * P))
    wv_b = (np.asarray(inputs["Wv"], np.float32).astype(bf)
            .reshape(NF, P, DOUT).transpose(1, 0, 2).reshape(P, NF * DOUT))
    ball_b = np.concatenate([bq, np.zeros(DOUT, np.float32)]).astype(
        np.float32)[:, None]
    bv_b = np.asarray(inputs["bv"], np.float32).astype(bf)
    com_base = np.zeros((P, CB), dtype=np.uint8)
    o = 0
    com_base[:, o:o + CB_WQK] = wqk_b.view(np.uint8); o += CB_WQK
    com_base[:, o:o + CB_WV] = wv_b.view(np.uint8); o += CB_WV
    com_base[:, o:o + CB_BALL] = ball_b.view(np.uint8); o += CB_BALL
    o_pit = o; o += CB_PIT
    com_base[0, o:o + CB_BV] = bv_b.view(np.uint8); o += CB_BV

    in_maps = []
    for c in range(N_CORES):
        b, h = c // 2, c % 2
        qsl = slice(h * H, (h + 1) * H)
        # key order rotated per core: [my 1024 keys, partner's 1024]
        nmT = nm[b, qsl, :].T
        nmt = np.concatenate([nmT[h * H:(h + 1) * H],
                              nmT[(1 - h) * H:(2 - h) * H]], axis=0)
        cb = com_base.copy()
        cb[0, o_pit:o_pit + CB_PIT] = np.array(
            [1 - h], dtype=np.uint32).view(np.uint8)
        in_maps.append({
            "xt": np.ascontiguousarray(x[b, qsl].T.astype(bf)),
            "nmt": np.ascontiguousarray(nmt),
            "cb": cb,
        })
    return in_maps


_NC_CACHE = {}


def _get_nc(unroll: int = 1, fake_cc: bool = False):
    key = (unroll, fake_cc)
    if key not in _NC_CACHE:
        _NC_CACHE[key] = build_attention_nc(unroll, fake_cc)
    return _NC_CACHE[key]


def kernel(**inputs) -> np.ndarray:
    nc = _get_nc()
    in_maps = shard_inputs(inputs)
    res = run_bass_kernel_spmd(nc, in_maps, core_ids=list(range(N_CORES)))
    out = np.empty((B, S, DOUT), dtype=np.float32)
    for c in range(N_CORES):
        b, h = c // 2, c % 2
        out[b, h * H:(h + 1) * H] = res.results[c]["out"]
    return out


# revision 7
# speedup vs baseline: 1.0555x; 1.0119x over previous
"""Trainium2 Bass kernel for a single-head attention module (v4).

reference math (fp32):
    q = x @ Wq + bq; k = x @ Wk + bk; v = x @ Wv + bv        # [B,S,64]
    scores = (q @ k.T) / sqrt(S)                             # [B,S,S]
    scores = where(mask, -1e9, scores)
    out = softmax(scores, -1) @ v                            # [B,S,64]

Sharding: 8 cores = (batch b = c//2) x (sequence half h = c%2); each core
owns 1024 rows; pairs exchange K.T and V' via pairwise AllGathers. Key
order is host-rotated to [my keys, partner keys] so local attention
never waits on the exchange.

v4 layout/engine plan:
- Host supplies x pre-transposed ([DIN, H] bf16 — layout prep, like the
  mask rotation) and all small constants packed into ONE [128, CB] byte
  block (a single DMA; HWDGE slots are 625ns each and serialize).
- bk is dropped: (q+bq)@bk is constant per query and cancels in the
  softmax normalization.  One [Wq'|Wk] stationary pass projects Q.T and
  K.T together; a full-width DVE tensor_scalar_add applies bq while
  copying psum->sbuf (K rows +0), and the K.T half moves to a base-0
  tile via SBUF->SBUF DMA (engines cannot cross partitions; DMA can).
  1/sqrt(S) is folded into Wq'/bq' on host.
- V is computed in natural [key, d] layout (x.T chunks stationary, Wv
  moving) so V' needs no transposes; bv enters via rank-1 ones-row
  matmuls; V' carries a ones-column that makes the C' matmul produce
  the softmax denominator for free.
- Attention per 128-key chunk: f32r score matmuls into a 2-bank psum,
  exp on ACT (psum->sbuf bf16), u8 keep-mask multiply, C'-accumulate
  matmuls (V' stationary bf16, P.T moving bf16), emitted two chunks
  behind the score matmuls so the PE queue never stalls the exp stream
  on a late mask multiply or readback.  DVE takes one multiply per
  chunk (n=0 plus every 4th n=1) so it never outpaces the 1038ns exp
  period; Pool absorbs the rest.
- The SP DMA queue is hand-ordered so the single DMA-device FIFO serves
  transfers roughly in deadline order: x group 0, mask 0-1, first half
  of x group 1, kt0 (ahead of the last x pairs), mask 2-3, the split
  exchange stages (kt0, kt1, V'), and the remaining mask chunks.
- Finalize per query group: one DVE psum->sbuf copy, then per-128-query
  PE transpose, DVE reciprocal of the denominator column, ACT multiply
  (idle after the last exp), one output DMA per group.
"""

import numpy as np
import ml_dtypes

import concourse.bass as bass
import concourse.mybir as mybir
import concourse.tile as tile
from concourse import bacc
from concourse.bass_utils import run_bass_kernel_spmd
from concourse.masks import make_identity
from concourse.tile import add_dep_helper

B, S, DIN, DOUT = 4, 2048, 1024, 64
H = S // 2          # rows (queries/keys) owned per core
P = 128             # partitions
NF = DIN // P       # 8 feature chunks
NS = S // P         # 16 key chunks (rotated order: 0-7 local, 8-15 partner)
QC = 512            # queries per projection group / matmul moving limit
NQC = H // QC       # 2 query groups
DP = DOUT + 1       # V' columns (V plus ones-column)
SINGLES = 0         # leading local chunks exp'd 512-wide per query group
KTG_BY = DOUT * QC * 4      # bytes of one K.T group (kept f32r)
VP_BY = P * NF * DP * 2     # bytes of local V' (bf16)
# packed constant block: wqk | wv | ball | pit | bv (bytes per partition)
CB_WQK = NF * P * 2
CB_WV = NF * DOUT * 2
CB_BALL = 4
CB_PIT = 4
CB_BV = DOUT * 2
CB = CB_WQK + CB_WV + CB_BALL + CB_PIT + CB_BV

F32 = mybir.dt.float32
F32R = mybir.dt.float32r
BF16 = mybir.dt.bfloat16
U8 = mybir.dt.uint8

N_CORES = 8
PAIRS = [[0, 1], [2, 3], [4, 5], [6, 7]]


def _pool_mask_half(ci, n, single):
    """True -> mask multiply for this (chunk, group) half runs on Pool.
    DVE gets one 594ns multiply per chunk (n=0) so it never outpaces the
    1038ns exp period; Pool takes most n=1 halves; every 4th chunk's n=1
    stays on DVE so Pool (1111ns/op) does not accumulate a backlog that
    would delay the final C' accumulations."""
    return n == 1 and (ci % 4 != 3 or ci == 15)


def build_attention_nc(unroll: int = 1, fake_cc: bool = False):
    nc = bacc.Bacc("TRN2", target_bir_lowering=False, debug=False,
                   num_devices=N_CORES)

    xt_d = nc.dram_tensor("xt", [DIN, H], BF16, kind="ExternalInput")
    nmt_d = nc.dram_tensor("nmt", [S, H], U8, kind="ExternalInput")
    cb_d = nc.dram_tensor("cb", [P, CB], U8, kind="ExternalInput")
    out_d = nc.dram_tensor("out", [H, DOUT], F32, kind="ExternalOutput")

    Exp = mybir.ActivationFunctionType.Exp

    with tile.TileContext(nc) as tc:
        with (
            tc.tile_pool(name="consts", bufs=1) as consts,
            tc.tile_pool(name="persist", bufs=1) as persist,
            tc.tile_pool(name="ptp", bufs=4) as ptp,
            tc.tile_pool(name="p2p", bufs=8) as p2p,
            tc.tile_pool(name="fin", bufs=4) as fin,
            tc.tile_pool(name="dramb", bufs=1, space="DRAM") as dramb,
            tc.tile_pool(name="st_ps", bufs=2, space="PSUM") as st_ps,
            tc.tile_pool(name="scr_ps", bufs=2, space="PSUM") as scr_ps,
            tc.tile_pool(name="cp_ps", bufs=1, space="PSUM") as cp_ps,
        ):
            # ---- packed constants (DMA emitted in the SP issue order) ------
            cbl = consts.tile([P, CB], U8, tag="cbl")
            o0 = 0
            wqk = cbl[:, o0:o0 + CB_WQK].bitcast(BF16).rearrange(
                "p (c d) -> p c d", d=P)
            o0 += CB_WQK
            wv = cbl[:, o0:o0 + CB_WV].bitcast(BF16).rearrange(
                "p (c d) -> p c d", d=DOUT)
            o0 += CB_WV
            ball = cbl[:, o0:o0 + CB_BALL].bitcast(F32)
            o0 += CB_BALL
            pit = cbl[0:1, o0:o0 + CB_PIT].bitcast(mybir.dt.uint32)
            o0 += CB_PIT
            bvrow = cbl[0:1, o0:o0 + CB_BV].bitcast(BF16)
            ones = consts.tile([1, P], BF16, tag="ones")
            nc.vector.memset(ones, 1.0)
            ident = consts.tile([P, P], F32, tag="ident")
            make_identity(nc, ident)
            # PE warmup: serial transpose chain ramps the tensor engine
            # p-state before the first projection matmuls
            pwarm = scr_ps.tile([P, QC], F32, tag="scr")
            for _ in range(17):
                nc.tensor.transpose(pwarm[:, :P], ident, ident)
            # preload the ACT Exp table so the first real exp skips the
            # 1283ns table load
            wtiny = consts.tile([1, 1], F32, tag="wtiny")
            nc.scalar.activation(out=wtiny, in_=ident[0:1, 0:1], func=Exp)

            for u in range(unroll):
                xt = persist.tile([P, NF, H], BF16, tag="xt", name="xt")
                nm8 = persist.tile([P, NS, H], U8, tag="m8", name="m8")
                qk = [
                    persist.tile([P, QC], F32R, tag=f"qk{g}", name=f"qk{g}")
                    for g in range(NQC)
                ]
                kt = [
                    persist.tile([DOUT, QC], F32R, tag=f"kt{g}", name=f"kt{g}")
                    for g in range(NQC)
                ]
                ktp = [
                    persist.tile([DOUT, QC], F32R, tag=f"ktp{g}",
                                 name=f"ktp{g}")
                    for g in range(NQC)
                ]
                vp = persist.tile([P, NF, DP], BF16, tag="vp", name="vp")
                vpp = persist.tile([P, NF, DP], BF16, tag="vpp", name="vpp")
                exi = [
                    dramb.tile([1, KTG_BY], U8, tag=f"exi{g}", name=f"exi{g}")
                    for g in range(NQC)
                ]
                exo = [
                    dramb.tile([2, KTG_BY], U8, tag=f"exo{g}", name=f"exo{g}")
                    for g in range(NQC)
                ]
                exvi = dramb.tile([1, VP_BY], U8, tag="exvi", name="exvi")
                exvo = dramb.tile([2, VP_BY], U8, tag="exvo", name="exvo")

                def xload(g, fp):
                    return nc.sync.dma_start(
                        out=xt[:, 2 * fp:2 * fp + 2, g * QC:(g + 1) * QC],
                        in_=xt_d.ap()[2 * fp * P:(2 * fp + 2) * P,
                                      g * QC:(g + 1) * QC].rearrange(
                            "(c p) s -> p c s", p=P),
                    )

                def mask_dma(eng, lo, hi):
                    return eng.dma_start(
                        out=nm8[:, lo:hi, :],
                        in_=nmt_d.ap()[lo * P:hi * P, :].rearrange(
                            "(c p) q -> p c q", p=P),
                    )

                def project_qk(g):
                    """[Wq'|Wk] pass for one 512-row group."""
                    pqk = scr_ps.tile([P, QC], F32, tag="scr")
                    for cf in range(NF):
                        nc.tensor.matmul(
                            pqk, wqk[:, cf], xt[:, cf, g * QC:(g + 1) * QC],
                            start=(cf == 0), stop=(cf == NF - 1),
                        )
                    # full-width copy applies bq (K rows get +0); on ACT,
                    # which idles until the first exp
                    nc.scalar.activation(
                        out=qk[g], in_=pqk,
                        func=mybir.ActivationFunctionType.Identity,
                        bias=ball)

                def project_v(g):
                    pv = scr_ps.tile([P, QC], F32, tag="scr")
                    for sb in range(4 * g, 4 * (g + 1)):
                        o = (sb - 4 * g) * DOUT
                        for cf in range(NF):
                            nc.tensor.matmul(
                                pv[:, o:o + DOUT],
                                xt[:, cf, sb * P:(sb + 1) * P],
                                wv[:, cf],
                                start=(cf == 0), stop=False,
                            )
                        nc.tensor.matmul(
                            pv[:, o:o + DOUT], ones, bvrow,
                            start=False, stop=True,
                        )
                    if g == 0:
                        nc.vector.memset(vp, 1.0)
                    nc.vector.tensor_copy(
                        out=vp[:, 4 * g:4 * (g + 1), :DOUT],
                        in_=pv[:, :4 * DOUT].rearrange(
                            "p (c d) -> p c d", d=DOUT),
                    )

                def exchange_kt(g):
                    nc.sync.dma_start(
                        out=exi[g][0:1, :].bitcast(F32R)
                        .rearrange("one (k s) -> k (one s)", k=DOUT),
                        in_=qk[g][DOUT:, :],
                    )
                    if fake_cc:
                        nc.sync.dma_start(out=exo[g][0], in_=exi[g][0])
                        nc.sync.dma_start(out=exo[g][1], in_=exi[g][0])
                    else:
                        nc.gpsimd.collective_compute(
                            "AllGather", mybir.AluOpType.bypass,
                            replica_groups=PAIRS,
                            ins=[exi[g][:]], outs=[exo[g][:]],
                        )

                def readback_kt(g):
                    nc.sync.dma_start(
                        out=ktp[g],
                        in_=exo[g][:].bitcast(F32R)
                        .rearrange("two (k s) -> two k s", k=DOUT)
                        [bass.ds(prv, 1), :, :]
                        .rearrange("one k s -> k (one s)"),
                    )

                # ---- attention emission helpers ---------------------------
                def chunk_views(ci):
                    if ci < NS // 2:
                        g, kb = ci // 4, (ci % 4) * P
                        return kt[g][:, kb:kb + P], vp[:, ci, :]
                    g, kb = (ci - 8) // 4, (ci % 4) * P
                    return ktp[g][:, kb:kb + P], vpp[:, ci - 8, :]

                cps = [
                    cp_ps.tile([DP, QC], F32, tag=f"cp{n}", name=f"cp{n}")
                    for n in range(NQC)
                ]
                sched = [(ci, (0, 1), False) for ci in range(NS - 1)]
                sched += [(NS - 1, (0,), True), (NS - 1, (1,), True)]
                first = {}
                last = {}
                for pos, (ci, n_list, single) in enumerate(sched):
                    for n in n_list:
                        first.setdefault(n, pos)
                        last[n] = pos

                def emit_attention(lo, hi):
                    # C' matmuls are emitted two chunks behind the score
                    # matmuls so the PE queue never stalls the exp stream on
                    # a late mask multiply or V'/K.T readback
                    pending = []

                    def flush_cprime():
                        pos, n, vp_sl, p2 = pending.pop(0)
                        nc.tensor.matmul(
                            cps[n], vp_sl, p2,
                            start=(pos == first[n]), stop=(pos == last[n]),
                        )

                    for pos in range(lo, hi):
                        ci, n_list, single = sched[pos]
                        kt_sl, vp_sl = chunk_views(ci)
                        st = st_ps.tile([P, H], F32, tag="st")
                        for n in n_list:
                            nc.tensor.matmul(
                                st[:, n * QC:(n + 1) * QC], kt_sl,
                                qk[n][:DOUT, :],
                                start=True, stop=True,
                            )
                        pt = ptp.tile([P, H], BF16, tag="pt")
                        if single:
                            n = n_list[0]
                            nc.scalar.activation(
                                out=pt[:, :QC],
                                in_=st[:, n * QC:(n + 1) * QC], func=Exp)
                        else:
                            nc.scalar.activation(out=pt, in_=st, func=Exp)
                        for n in n_list:
                            psl = slice(0, QC) if single else slice(
                                n * QC, (n + 1) * QC)
                            p2 = p2p.tile([P, QC], BF16, tag=f"p2_{n}",
                                          name=f"p2_{n}")
                            eng = (nc.gpsimd if _pool_mask_half(ci, n, single)
                                   else nc.vector)
                            eng.tensor_mul(p2, pt[:, psl],
                                           nm8[:, ci, n * QC:(n + 1) * QC])
                            pending.append((pos, n, vp_sl, p2))
                        while len(pending) > 8:
                            flush_cprime()
                    while pending:
                        flush_cprime()

                # ---- issue order (SP queue == DMA deadline order) ---------
                xload(0, 0)
                if u == 0:
                    nc.sync.dma_start(out=cbl, in_=cb_d.ap())
                    pregs = nc.alloc_registers()
                for fp in range(1, NF // 2):
                    xload(0, fp)
                for fp in range(NF // 2):
                    xload(1, fp)
                project_qk(0)
                project_v(0)
                nc.sync.dma_start(out=kt[0], in_=qk[0][DOUT:, :])
                mask_dma(nc.sync, 0, 2)
                mask_dma(nc.sync, 2, 4)
                if u == 0:
                    nc.regs_load(pregs, pit)
                    prv = nc.snap(pregs)
                project_qk(1)
                project_v(1)
                nc.sync.dma_start(out=kt[1], in_=qk[1][DOUT:, :])
                nc.sync.dma_start(
                    out=exi[0][0:1, :].bitcast(F32R)
                    .rearrange("one (k s) -> k (one s)", k=DOUT),
                    in_=qk[0][DOUT:, :],
                )
                if fake_cc:
                    nc.sync.dma_start(out=exo[0][0], in_=exi[0][0])
                    nc.sync.dma_start(out=exo[0][1], in_=exi[0][0])
                else:
                    nc.gpsimd.collective_compute(
                        "AllGather", mybir.AluOpType.bypass,
                        replica_groups=PAIRS,
                        ins=[exi[0][:]], outs=[exo[0][:]],
                    )
                mask_dma(nc.sync, 4, 6)
                nc.sync.dma_start(
                    out=exi[1][0:1, :].bitcast(F32R)
                    .rearrange("one (k s) -> k (one s)", k=DOUT),
                    in_=qk[1][DOUT:, :],
                )
                readback_kt(0)
                mask_dma(nc.sync, 6, 8)
                mask_dma(nc.sync, 8, 10)
                nc.sync.dma_start(
                    out=exvi[0:1, :].bitcast(BF16).rearrange(
                        "one (p d) -> p (one d)", p=P),
                    in_=vp[:].rearrange("p c d -> p (c d)"),
                )
                if fake_cc:
                    nc.sync.dma_start(out=exo[1][0], in_=exi[1][0])
                    nc.sync.dma_start(out=exo[1][1], in_=exi[1][0])
                    nc.sync.dma_start(out=exvo[0], in_=exvi[0])
                    nc.sync.dma_start(out=exvo[1], in_=exvi[0])
                else:
                    nc.gpsimd.collective_compute(
                        "AllGather", mybir.AluOpType.bypass,
                        replica_groups=PAIRS,
                        ins=[exi[1][:]], outs=[exo[1][:]],
                    )
                    nc.gpsimd.collective_compute(
                        "AllGather", mybir.AluOpType.bypass,
                        replica_groups=PAIRS,
                        ins=[exvi[:]], outs=[exvo[:]],
                    )
                readback_kt(1)
                mask_dma(nc.sync, 10, 12)
                nc.sync.dma_start(
                    out=vpp[:].rearrange("p c d -> p (c d)"),
                    in_=exvo[:].bitcast(BF16)
                    .rearrange("two (p d) -> two p d", p=P)
                    [bass.ds(prv, 1), :, :]
                    .rearrange("one p d -> p (one d)"),
                )
                mask_dma(nc.sync, 12, NS)
                emit_attention(SINGLES, len(sched))

                # ---- finalize: one copy + one DMA per group ---------------
                for n in range(NQC):
                    ct = fin.tile([DP, QC], F32, tag="ct")
                    nc.vector.tensor_copy(out=ct, in_=cps[n])
                    c_sb = fin.tile([P, QC // P, DOUT], F32, tag="c_sb")
                    for qb in range(QC // P):
                        tp = scr_ps.tile([P, QC], F32, tag="scr")
                        nc.tensor.transpose(
                            tp[:, :DP], ct[:, qb * P:(qb + 1) * P],
                            ident[:DP, :DP])
                        rec = fin.tile([P, 1], F32, tag="rec")
                        nc.vector.reciprocal(rec, tp[:, DOUT:DP])
                        nc.scalar.mul(c_sb[:, qb, :], tp[:, :DOUT], rec)
                    nc.sync.dma_start(
                        out=out_d.ap()[n * QC:(n + 1) * QC, :].rearrange(
                            "(c p) d -> p c d", p=P),
                        in_=c_sb,
                    )

    nc.compile()
    return nc


def shard_inputs(inputs):
    """Full inputs -> per-core in_maps (list of 8 dicts)."""
    bf = ml_dtypes.bfloat16
    x = np.asarray(inputs["input_tensor"], dtype=np.float32)
    m = np.asarray(inputs["attention_mask"])
    nm = (~m).view(np.uint8) if m.dtype == np.bool_ else (m == 0).astype(np.uint8)

    scale = np.float32(np.sqrt(np.float32(S)))
    wq = np.asarray(inputs["Wq"], np.float32) / scale
    bq = np.asarray(inputs["bq"], np.float32) / scale
    wk = np.asarray(inputs["Wk"], np.float32)
    # bk is omitted: it only shifts scores by a per-query constant, which
    # softmax normalization cancels.
    wqk_b = (np.concatenate([wq, wk], axis=1).astype(bf)
             .reshape(NF, P, P).transpose(1, 0, 2).reshape(P, NF * P))
    wv_b = (np.asarray(inputs["Wv"], np.float32).astype(bf)
            .reshape(NF, P, DOUT).transpose(1, 0, 2).reshape(P, NF * DOUT))
    ball_b = np.concatenate([bq, np.zeros(DOUT, np.float32)]).astype(
        np.float32)[:, None]
    bv_b = np.asarray(inputs["bv"], np.float32).astype(bf)
    com_base = np.zeros((P, CB), dtype=np.uint8)
    o = 0
    com_base[:, o:o + CB_WQK] = wqk_b.view(np.uint8); o += CB_WQK
    com_base[:, o:o + CB_WV] = wv_b.view(np.uint8); o += CB_WV
    com_base[:, o:o + CB_BALL] = ball_b.view(np.uint8); o += CB_BALL
    o_pit = o; o += CB_PIT
    com_base[0, o:o + CB_BV] = bv_b.view(np.uint8); o += CB_BV

    in_maps = []
    for c in range(N_CORES):
        b, h = c // 2, c % 2
        qsl = slice(h * H, (h + 1) * H)
        # key order rotated per core: [my 1024 keys, partner's 1024]
        nmT = nm[b, qsl, :].T
        nmt = np.concatenate([nmT[h * H:(h + 1) * H],
                              nmT[(1 - h) * H:(2 - h) * H]], axis=0)
        cb = com_base.copy()
        cb[0, o_pit:o_pit + CB_PIT] = np.array(
            [1 - h], dtype=np.uint32).view(np.uint8)
        in_maps.append({
            "xt": np.ascontiguousarray(x[b, qsl].T.astype(bf)),
            "nmt": np.ascontiguousarray(nmt),
            "cb": cb,
        })
    return in_maps


_NC_CACHE = {}


def _get_nc(unroll: int = 1, fake_cc: bool = False):
    key = (unroll, fake_cc)
    if key not in _NC_CACHE:
        _NC_CACHE[key] = build_attention_nc(unroll, fake_cc)
    return _NC_CACHE[key]


def kernel(**inputs) -> np.ndarray:
    nc = _get_nc()
    in_maps = shard_inputs(inputs)
    res = run_bass_kernel_spmd(nc, in_maps, core_ids=list(range(N_CORES)))
    out = np.empty((B, S, DOUT), dtype=np.float32)
    for c in range(N_CORES):
        b, h = c // 2, c % 2
        out[b, h * H:(h + 1) * H] = res.results[c]["out"]
    return out


# revision 8
# speedup vs baseline: 1.0592x; 1.0036x over previous
"""Trainium2 Bass kernel for a single-head attention module (v4).

reference math (fp32):
    q = x @ Wq + bq; k = x @ Wk + bk; v = x @ Wv + bv        # [B,S,64]
    scores = (q @ k.T) / sqrt(S)                             # [B,S,S]
    scores = where(mask, -1e9, scores)
    out = softmax(scores, -1) @ v                            # [B,S,64]

Sharding: 8 cores = (batch b = c//2) x (sequence half h = c%2); each core
owns 1024 rows; pairs exchange K.T and V' via pairwise AllGathers. Key
order is host-rotated to [my keys, partner keys] so local attention
never waits on the exchange.

v4 layout/engine plan:
- Host supplies x pre-transposed ([DIN, H] bf16 — layout prep, like the
  mask rotation) and all small constants packed into ONE [128, CB] byte
  block (a single DMA; HWDGE slots are 625ns each and serialize).
- bk is dropped: (q+bq)@bk is constant per query and cancels in the
  softmax normalization.  One [Wq'|Wk] stationary pass projects Q.T and
  K.T together; a full-width DVE tensor_scalar_add applies bq while
  copying psum->sbuf (K rows +0), and the K.T half moves to a base-0
  tile via SBUF->SBUF DMA (engines cannot cross partitions; DMA can).
  1/sqrt(S) is folded into Wq'/bq' on host.
- V is computed in natural [key, d] layout (x.T chunks stationary, Wv
  moving) so V' needs no transposes; bv enters via rank-1 ones-row
  matmuls; V' carries a ones-column that makes the C' matmul produce
  the softmax denominator for free.
- Attention per 128-key chunk: f32r score matmuls into a 2-bank psum,
  exp on ACT (psum->sbuf bf16), u8 keep-mask multiply, C'-accumulate
  matmuls (V' stationary bf16, P.T moving bf16), emitted two chunks
  behind the score matmuls so the PE queue never stalls the exp stream
  on a late mask multiply or readback.  DVE takes one multiply per
  chunk (n=0 plus every 4th n=1) so it never outpaces the 1038ns exp
  period; Pool absorbs the rest.
- The SP DMA queue is hand-ordered so the single DMA-device FIFO serves
  transfers roughly in deadline order: x group 0, mask 0-1, first half
  of x group 1, kt0 (ahead of the last x pairs), mask 2-3, the split
  exchange stages (kt0, kt1, V'), and the remaining mask chunks.
- Finalize per query group: one DVE psum->sbuf copy, then per-128-query
  PE transpose, DVE reciprocal of the denominator column, ACT multiply
  (idle after the last exp), one output DMA per group.
"""

import numpy as np
import ml_dtypes

import concourse.bass as bass
import concourse.mybir as mybir
import concourse.tile as tile
from concourse import bacc
from concourse.bass_utils import run_bass_kernel_spmd
from concourse.masks import make_identity
from concourse.tile import add_dep_helper

B, S, DIN, DOUT = 4, 2048, 1024, 64
H = S // 2          # rows (queries/keys) owned per core
P = 128             # partitions
NF = DIN // P       # 8 feature chunks
NS = S // P         # 16 key chunks (rotated order: 0-7 local, 8-15 partner)
QC = 512            # queries per projection group / matmul moving limit
NQC = H // QC       # 2 query groups
DP = DOUT + 1       # V' columns (V plus ones-column)
SINGLES = 0         # leading local chunks exp'd 512-wide per query group
KTG_BY = DOUT * QC * 4      # bytes of one K.T group (kept f32r)
VP_BY = P * NF * DP * 2     # bytes of local V' (bf16)
# packed constant block: wqk | wv | ball | pit | bv (bytes per partition)
CB_WQK = NF * P * 2
CB_WV = NF * DOUT * 2
CB_BALL = 4
CB_PIT = 4
CB_BV = DOUT * 2
CB = CB_WQK + CB_WV + CB_BALL + CB_PIT + CB_BV

F32 = mybir.dt.float32
F32R = mybir.dt.float32r
BF16 = mybir.dt.bfloat16
U8 = mybir.dt.uint8

N_CORES = 8
PAIRS = [[0, 1], [2, 3], [4, 5], [6, 7]]


def _pool_mask_half(ci, n, single):
    """True -> mask multiply for this (chunk, group) half runs on Pool.
    DVE gets one 594ns multiply per chunk (n=0) so it never outpaces the
    1038ns exp period; Pool takes most n=1 halves; every 4th chunk's n=1
    stays on DVE so Pool (1111ns/op) does not accumulate a backlog that
    would delay the final C' accumulations."""
    return n == 1 and (ci % 4 != 3 or ci == 15)


def build_attention_nc(unroll: int = 1, fake_cc: bool = False):
    nc = bacc.Bacc("TRN2", target_bir_lowering=False, debug=False,
                   num_devices=N_CORES)

    xt_d = nc.dram_tensor("xt", [DIN, H], BF16, kind="ExternalInput")
    nmt_d = nc.dram_tensor("nmt", [S, H], U8, kind="ExternalInput")
    cb_d = nc.dram_tensor("cb", [P, CB], U8, kind="ExternalInput")
    out_d = nc.dram_tensor("out", [H, DOUT], F32, kind="ExternalOutput")

    Exp = mybir.ActivationFunctionType.Exp

    with tile.TileContext(nc) as tc:
        with (
            tc.tile_pool(name="consts", bufs=1) as consts,
            tc.tile_pool(name="persist", bufs=1) as persist,
            tc.tile_pool(name="ptp", bufs=4) as ptp,
            tc.tile_pool(name="p2p", bufs=8) as p2p,
            tc.tile_pool(name="fin", bufs=4) as fin,
            tc.tile_pool(name="dramb", bufs=1, space="DRAM") as dramb,
            tc.tile_pool(name="st_ps", bufs=2, space="PSUM") as st_ps,
            tc.tile_pool(name="scr_ps", bufs=2, space="PSUM") as scr_ps,
            tc.tile_pool(name="cp_ps", bufs=1, space="PSUM") as cp_ps,
        ):
            # ---- packed constants (DMA emitted in the SP issue order) ------
            cbl = consts.tile([P, CB], U8, tag="cbl")
            o0 = 0
            wqk = cbl[:, o0:o0 + CB_WQK].bitcast(BF16).rearrange(
                "p (c d) -> p c d", d=P)
            o0 += CB_WQK
            wv = cbl[:, o0:o0 + CB_WV].bitcast(BF16).rearrange(
                "p (c d) -> p c d", d=DOUT)
            o0 += CB_WV
            ball = cbl[:, o0:o0 + CB_BALL].bitcast(F32)
            o0 += CB_BALL
            pit = cbl[0:1, o0:o0 + CB_PIT].bitcast(mybir.dt.uint32)
            o0 += CB_PIT
            bvrow = cbl[0:1, o0:o0 + CB_BV].bitcast(BF16)
            ones = consts.tile([1, P], BF16, tag="ones")
            nc.vector.memset(ones, 1.0)
            ident = consts.tile([P, P], F32, tag="ident")
            make_identity(nc, ident)
            # PE warmup: serial transpose chain ramps the tensor engine
            # p-state before the first projection matmuls
            pwarm = scr_ps.tile([P, QC], F32, tag="scr")
            for _ in range(13):
                nc.tensor.transpose(pwarm[:, :P], ident, ident)
            # preload the ACT Exp table so the first real exp skips the
            # 1283ns table load
            wtiny = consts.tile([1, 1], F32, tag="wtiny")
            nc.scalar.activation(out=wtiny, in_=ident[0:1, 0:1], func=Exp)

            for u in range(unroll):
                xt = persist.tile([P, NF, H], BF16, tag="xt", name="xt")
                nm8 = persist.tile([P, NS, H], U8, tag="m8", name="m8")
                qk = [
                    persist.tile([P, QC], F32R, tag=f"qk{g}", name=f"qk{g}")
                    for g in range(NQC)
                ]
                kt = [
                    persist.tile([DOUT, QC], F32R, tag=f"kt{g}", name=f"kt{g}")
                    for g in range(NQC)
                ]
                ktp = [
                    persist.tile([DOUT, QC], F32R, tag=f"ktp{g}",
                                 name=f"ktp{g}")
                    for g in range(NQC)
                ]
                vp = persist.tile([P, NF, DP], BF16, tag="vp", name="vp")
                vpp = persist.tile([P, NF, DP], BF16, tag="vpp", name="vpp")
                exi = [
                    dramb.tile([1, KTG_BY], U8, tag=f"exi{g}", name=f"exi{g}")
                    for g in range(NQC)
                ]
                exo = [
                    dramb.tile([2, KTG_BY], U8, tag=f"exo{g}", name=f"exo{g}")
                    for g in range(NQC)
                ]
                exvi = dramb.tile([1, VP_BY], U8, tag="exvi", name="exvi")
                exvo = dramb.tile([2, VP_BY], U8, tag="exvo", name="exvo")

                def xload(g, fp):
                    return nc.sync.dma_start(
                        out=xt[:, 2 * fp:2 * fp + 2, g * QC:(g + 1) * QC],
                        in_=xt_d.ap()[2 * fp * P:(2 * fp + 2) * P,
                                      g * QC:(g + 1) * QC].rearrange(
                            "(c p) s -> p c s", p=P),
                    )

                def mask_dma(eng, lo, hi):
                    return eng.dma_start(
                        out=nm8[:, lo:hi, :],
                        in_=nmt_d.ap()[lo * P:hi * P, :].rearrange(
                            "(c p) q -> p c q", p=P),
                    )

                def project_qk(g):
                    """[Wq'|Wk] pass for one 512-row group."""
                    pqk = scr_ps.tile([P, QC], F32, tag="scr")
                    for cf in range(NF):
                        nc.tensor.matmul(
                            pqk, wqk[:, cf], xt[:, cf, g * QC:(g + 1) * QC],
                            start=(cf == 0), stop=(cf == NF - 1),
                        )
                    # full-width copy applies bq (K rows get +0); on ACT,
                    # which idles until the first exp
                    nc.scalar.activation(
                        out=qk[g], in_=pqk,
                        func=mybir.ActivationFunctionType.Identity,
                        bias=ball)

                def project_v(g):
                    pv = scr_ps.tile([P, QC], F32, tag="scr")
                    for sb in range(4 * g, 4 * (g + 1)):
                        o = (sb - 4 * g) * DOUT
                        for cf in range(NF):
                            nc.tensor.matmul(
                                pv[:, o:o + DOUT],
                                xt[:, cf, sb * P:(sb + 1) * P],
                                wv[:, cf],
                                start=(cf == 0), stop=False,
                            )
                        nc.tensor.matmul(
                            pv[:, o:o + DOUT], ones, bvrow,
                            start=False, stop=True,
                        )
                    if g == 0:
                        nc.vector.memset(vp, 1.0)
                    nc.vector.tensor_copy(
                        out=vp[:, 4 * g:4 * (g + 1), :DOUT],
                        in_=pv[:, :4 * DOUT].rearrange(
                            "p (c d) -> p c d", d=DOUT),
                    )

                def exchange_kt(g):
                    nc.sync.dma_start(
                        out=exi[g][0:1, :].bitcast(F32R)
                        .rearrange("one (k s) -> k (one s)", k=DOUT),
                        in_=qk[g][DOUT:, :],
                    )
                    if fake_cc:
                        nc.sync.dma_start(out=exo[g][0], in_=exi[g][0])
                        nc.sync.dma_start(out=exo[g][1], in_=exi[g][0])
                    else:
                        nc.gpsimd.collective_compute(
                            "AllGather", mybir.AluOpType.bypass,
                            replica_groups=PAIRS,
                            ins=[exi[g][:]], outs=[exo[g][:]],
                        )

                def readback_kt(g):
                    nc.sync.dma_start(
                        out=ktp[g],
                        in_=exo[g][:].bitcast(F32R)
                        .rearrange("two (k s) -> two k s", k=DOUT)
                        [bass.ds(prv, 1), :, :]
                        .rearrange("one k s -> k (one s)"),
                    )

                # ---- attention emission helpers ---------------------------
                def chunk_views(ci):
                    if ci < NS // 2:
                        g, kb = ci // 4, (ci % 4) * P
                        return kt[g][:, kb:kb + P], vp[:, ci, :]
                    g, kb = (ci - 8) // 4, (ci % 4) * P
                    return ktp[g][:, kb:kb + P], vpp[:, ci - 8, :]

                cps = [
                    cp_ps.tile([DP, QC], F32, tag=f"cp{n}", name=f"cp{n}")
                    for n in range(NQC)
                ]
                sched = [(ci, (0, 1), False) for ci in range(NS - 1)]
                sched += [(NS - 1, (0,), True), (NS - 1, (1,), True)]
                first = {}
                last = {}
                for pos, (ci, n_list, single) in enumerate(sched):
                    for n in n_list:
                        first.setdefault(n, pos)
                        last[n] = pos

                def emit_attention(lo, hi):
                    # C' matmuls are emitted two chunks behind the score
                    # matmuls so the PE queue never stalls the exp stream on
                    # a late mask multiply or V'/K.T readback
                    pending = []

                    def flush_cprime():
                        pos, n, vp_sl, p2 = pending.pop(0)
                        nc.tensor.matmul(
                            cps[n], vp_sl, p2,
                            start=(pos == first[n]), stop=(pos == last[n]),
                        )

                    for pos in range(lo, hi):
                        ci, n_list, single = sched[pos]
                        kt_sl, vp_sl = chunk_views(ci)
                        st = st_ps.tile([P, H], F32, tag="st")
                        for n in n_list:
                            nc.tensor.matmul(
                                st[:, n * QC:(n + 1) * QC], kt_sl,
                                qk[n][:DOUT, :],
                                start=True, stop=True,
                            )
                        pt = ptp.tile([P, H], BF16, tag="pt")
                        if single:
                            n = n_list[0]
                            nc.scalar.activation(
                                out=pt[:, :QC],
                                in_=st[:, n * QC:(n + 1) * QC], func=Exp)
                        else:
                            nc.scalar.activation(out=pt, in_=st, func=Exp)
                        for n in n_list:
                            psl = slice(0, QC) if single else slice(
                                n * QC, (n + 1) * QC)
                            p2 = p2p.tile([P, QC], BF16, tag=f"p2_{n}",
                                          name=f"p2_{n}")
                            eng = (nc.gpsimd if _pool_mask_half(ci, n, single)
                                   else nc.vector)
                            eng.tensor_mul(p2, pt[:, psl],
                                           nm8[:, ci, n * QC:(n + 1) * QC])
                            pending.append((pos, n, vp_sl, p2))
                        while len(pending) > 8:
                            flush_cprime()
                    while pending:
                        flush_cprime()

                # ---- issue order (SP queue == DMA deadline order) ---------
                xload(0, 0)
                if u == 0:
                    nc.sync.dma_start(out=cbl, in_=cb_d.ap())
                    pregs = nc.alloc_registers()
                for fp in range(1, NF // 2):
                    xload(0, fp)
                for fp in range(NF // 2):
                    xload(1, fp)
                project_qk(0)
                project_v(0)
                nc.sync.dma_start(out=kt[0], in_=qk[0][DOUT:, :])
                mask_dma(nc.sync, 0, 2)
                mask_dma(nc.sync, 2, 4)
                if u == 0:
                    nc.regs_load(pregs, pit)
                    prv = nc.snap(pregs)
                project_qk(1)
                project_v(1)
                nc.sync.dma_start(out=kt[1], in_=qk[1][DOUT:, :])
                nc.sync.dma_start(
                    out=exi[0][0:1, :].bitcast(F32R)
                    .rearrange("one (k s) -> k (one s)", k=DOUT),
                    in_=qk[0][DOUT:, :],
                )
                if fake_cc:
                    nc.sync.dma_start(out=exo[0][0], in_=exi[0][0])
                    nc.sync.dma_start(out=exo[0][1], in_=exi[0][0])
                else:
                    nc.gpsimd.collective_compute(
                        "AllGather", mybir.AluOpType.bypass,
                        replica_groups=PAIRS,
                        ins=[exi[0][:]], outs=[exo[0][:]],
                    )
                mask_dma(nc.sync, 4, 6)
                nc.sync.dma_start(
                    out=exi[1][0:1, :].bitcast(F32R)
                    .rearrange("one (k s) -> k (one s)", k=DOUT),
                    in_=qk[1][DOUT:, :],
                )
                readback_kt(0)
                mask_dma(nc.sync, 6, 8)
                mask_dma(nc.sync, 8, 10)
                nc.sync.dma_start(
                    out=exvi[0:1, :].bitcast(BF16).rearrange(
                        "one (p d) -> p (one d)", p=P),
                    in_=vp[:].rearrange("p c d -> p (c d)"),
                )
                if fake_cc:
                    nc.sync.dma_start(out=exo[1][0], in_=exi[1][0])
                    nc.sync.dma_start(out=exo[1][1], in_=exi[1][0])
                    nc.sync.dma_start(out=exvo[0], in_=exvi[0])
                    nc.sync.dma_start(out=exvo[1], in_=exvi[0])
                else:
                    nc.gpsimd.collective_compute(
                        "AllGather", mybir.AluOpType.bypass,
                        replica_groups=PAIRS,
                        ins=[exi[1][:]], outs=[exo[1][:]],
                    )
                    nc.gpsimd.collective_compute(
                        "AllGather", mybir.AluOpType.bypass,
                        replica_groups=PAIRS,
                        ins=[exvi[:]], outs=[exvo[:]],
                    )
                readback_kt(1)
                mask_dma(nc.sync, 10, 12)
                nc.sync.dma_start(
                    out=vpp[:].rearrange("p c d -> p (c d)"),
                    in_=exvo[:].bitcast(BF16)
                    .rearrange("two (p d) -> two p d", p=P)
                    [bass.ds(prv, 1), :, :]
                    .rearrange("one p d -> p (one d)"),
                )
                mask_dma(nc.sync, 12, NS)
                emit_attention(SINGLES, len(sched))

                # ---- finalize: one copy, 4 transposes into ONE psum tile,
                # one strided 4-wide reciprocal, 4 ACT muls, one DMA ---------
                for n in range(NQC):
                    ct = fin.tile([DP, QC], F32, tag="ct")
                    nc.vector.tensor_copy(out=ct, in_=cps[n])
                    tp = scr_ps.tile([P, QC], F32, tag="scr")
                    for qb in range(QC // P):
                        nc.tensor.transpose(
                            tp[:, qb * DP:(qb + 1) * DP],
                            ct[:, qb * P:(qb + 1) * P],
                            ident[:DP, :DP])
                    rec = fin.tile([P, QC // P], F32, tag="rec")
                    nc.vector.reciprocal(
                        rec, tp[:, :(QC // P) * DP].rearrange(
                            "p (c d) -> p c d", d=DP)[:, :, DOUT])
                    c_sb = fin.tile([P, QC // P, DOUT], F32, tag="c_sb")
                    for qb in range(QC // P):
                        nc.scalar.mul(
                            c_sb[:, qb, :], tp[:, qb * DP:qb * DP + DOUT],
                            rec[:, qb:qb + 1])
                    nc.sync.dma_start(
                        out=out_d.ap()[n * QC:(n + 1) * QC, :].rearrange(
                            "(c p) d -> p c d", p=P),
                        in_=c_sb,
                    )

    nc.compile()
    return nc


def shard_inputs(inputs):
    """Full inputs -> per-core in_maps (list of 8 dicts)."""
    bf = ml_dtypes.bfloat16
    x = np.asarray(inputs["input_tensor"], dtype=np.float32)
    m = np.asarray(inputs["attention_mask"])
    nm = (~m).view(np.uint8) if m.dtype == np.bool_ else (m == 0).astype(np.uint8)

    scale = np.float32(np.sqrt(np.float32(S)))
    wq = np.asarray(inputs["Wq"], np.float32) / scale
    bq = np.asarray(inputs["bq"], np.float32) / scale
    wk = np.asarray(inputs["Wk"], np.float32)
    # bk is omitted: it only shifts scores by a per-query constant, which
    # softmax normalization cancels.
    wqk_b = (np.concatenate([wq, wk], axis=1).astype(bf)
             .reshape(NF, P, P).transpose(1, 0, 2).reshape(P, NF * P))
    wv_b = (np.asarray(inputs["Wv"], np.float32).astype(bf)
            .reshape(NF, P, DOUT).transpose(1, 0, 2).reshape(P, NF * DOUT))
    ball_b = np.concatenate([bq, np.zeros(DOUT, np.float32)]).astype(
        np.float32)[:, None]
    bv_b = np.asarray(inputs["bv"], np.float32).astype(bf)
    com_base = np.zeros((P, CB), dtype=np.uint8)
    o = 0
    com_base[:, o:o + CB_WQK] = wqk_b.view(np.uint8); o += CB_WQK
    com_base[:, o:o + CB_WV] = wv_b.view(np.uint8); o += CB_WV
    com_base[:, o:o + CB_BALL] = ball_b.view(np.uint8); o += CB_BALL
    o_pit = o; o += CB_PIT
    com_base[0, o:o + CB_BV] = bv_b.view(np.uint8); o += CB_BV

    in_maps = []
    for c in range(N_CORES):
        b, h = c // 2, c % 2
        qsl = slice(h * H, (h + 1) * H)
        # key order rotated per core: [my 1024 keys, partner's 1024]
        nmT = nm[b, qsl, :].T
        nmt = np.concatenate([nmT[h * H:(h + 1) * H],
                              nmT[(1 - h) * H:(2 - h) * H]], axis=0)
        cb = com_base.copy()
        cb[0, o_pit:o_pit + CB_PIT] = np.array(
            [1 - h], dtype=np.uint32).view(np.uint8)
        in_maps.append({
            "xt": np.ascontiguousarray(x[b, qsl].T.astype(bf)),
            "nmt": np.ascontiguousarray(nmt),
            "cb": cb,
        })
    return in_maps


_NC_CACHE = {}


def _get_nc(unroll: int = 1, fake_cc: bool = False):
    key = (unroll, fake_cc)
    if key not in _NC_CACHE:
        _NC_CACHE[key] = build_attention_nc(unroll, fake_cc)
    return _NC_CACHE[key]


def kernel(**inputs) -> np.ndarray:
    nc = _get_nc()
    in_maps = shard_inputs(inputs)
    res = run_bass_kernel_spmd(nc, in_maps, core_ids=list(range(N_CORES)))
    out = np.empty((B, S, DOUT), dtype=np.float32)
    for c in range(N_CORES):
        b, h = c // 2, c % 2
        out[b, h * H:(h + 1) * H] = res.results[c]["out"]
    return out
